# revision 1
# baseline (speedup 1.0000x reference)
"""Trainium2 Bass kernel: grouped (depthwise) time-domain cross-correlation.

Problem: data1, data2 [nb=32, nc=64, nt=8192] f32.
out[b,c,l] = sum_t data2[b,c,t] * data1[b,c, t + l - 257]  (data1 zero-padded),
l in [0, 515).   (== F.conv1d groups=nb*nc, padding=257)

Sharding: data-parallel over nb across 8 NeuronCores (4 nb-rows = 256 pairs each).

Per-core algorithm (per pair):
  Stage 1 (PE):  x2 chunk-matrix B[q,p] = x2[128q+p] as stationary (K=64,
    col-sliced per 32-phase-class beta); moving = x1 chunk rows shifted by g
    chunks.  22 matmuls write PSUM PP[32b+m, w] at free-offset
    o(g,b) = 128g - 32b + 257 so that PP[32b+m, w] accumulates lag-partials
    with  l = w - m  (shear only in m = phase mod 32).  Exact tiling: every
    PSUM cell written exactly once (start=True everywhere is safe).
  Stage 2 (DVE+ACT): drain PP -> SBUF.
  Stage 3 (PE): constant class matrix C32[p,m] = (p%32==m) reduces the 4
    beta-classes:  R[m,w] = sum_b PP[32b+m, w].
  Stage 4-6: R -> DRAM scratch (row stride 576) -> skewed re-read with
    partition stride 577 gives SK[m,l] = R[m, l+m]  (the shear fix; DRAM-side
    access patterns may step 1 element per partition, on-chip ones cannot).
  Stage 7 (PE): constant block-ones BD4[p,j] = (p//32==j) sums the 32 phases
    for 4 pairs at once: out4[j,l] = sum_m SK_j[m,l].
"""

import ml_dtypes
import numpy as np

import concourse.bacc as bacc
import concourse.bass as bass
import concourse.mybir as mybir
import concourse.tile as tile
from concourse.bass_utils import run_bass_kernel_spmd

# ---------------- problem constants (hardcoded per spec) ----------------
NB, NCH, NT = 32, 64, 8192
N_CORES = 8
NB_PER_CORE = NB // N_CORES          # 4
PAIRS = NB_PER_CORE * NCH            # 256 pairs per core
OUT_LEN = 515                        # 2*256 + 3
W = 548                              # w = l + m grid, even boundaries
XPAD = 384                           # x1 index range [-384, 8575]
X1LEN = NT + 2 * XPAD                # 8960
SCR_STRIDE = 576                     # scratch row stride (elements)
GROUPS = PAIRS // 4                  # 64 groups of 4 pairs

F32 = mybir.dt.float32
BF16 = mybir.dt.bfloat16

PSUM_BANK_ELEMS = 512  # fp32 per psum bank


def _mm_blocks():
    """Per beta: (b, g, c_lo, c_hi, w_lo) blocks tiling w in [0, W) exactly,
    with EVEN widths (f32r matmul needs even moving width) and a split at the
    psum bank boundary (w=512)."""
    blocks = []
    for b in range(4):
        for g in range(-4, 4):
            o = 128 * g - 32 * b + 257          # odd
            lo = max(0, o + 1)                  # even boundaries
            hi = min(W, o + 129)
            if hi <= lo:
                continue
            for a, z in ((lo, min(hi, PSUM_BANK_ELEMS)),
                         (max(lo, PSUM_BANK_ELEMS), hi)):
                if z > a:
                    blocks.append((b, g, a - o, z - o, a))
    return blocks


MM_BLOCKS = _mm_blocks()

# single wide x1 moving tile: A5[q, j] = x1[128q + j - A5OFF], j in [0, A5LEN)
A5OFF = 257
A5LEN = 644


def _consts():
    c32 = np.zeros((128, 32), np.float32)
    for p in range(128):
        c32[p, p % 32] = 1.0
    bd4 = np.zeros((128, 4), np.float32)
    for p in range(128):
        bd4[p, p // 32] = 1.0
    return c32, bd4


def _build(nc: bass.Bass):
    d1p = nc.dram_tensor("d1p", [PAIRS, X1LEN], BF16, kind="ExternalInput")
    d2 = nc.dram_tensor("d2", [PAIRS, NT], BF16, kind="ExternalInput")
    out = nc.dram_tensor("out", [PAIRS, OUT_LEN], F32, kind="ExternalOutput")

    c32_np, bd4_np = _consts()
    c32_dram = nc.inline_tensor(c32_np.astype(ml_dtypes.bfloat16), name="c32")
    bd4_dram = nc.inline_tensor(bd4_np.astype(ml_dtypes.bfloat16), name="bd4")

    with tile.TileContext(nc) as tc:
        with (
            tc.tile_pool(name="consts", bufs=1) as consts,
            tc.tile_pool(name="x2p", bufs=3) as x2p,
            tc.tile_pool(name="apool", bufs=3) as apool,
            tc.tile_pool(name="pp", bufs=2, space="PSUM") as pp_pool,
            tc.tile_pool(name="ppsb", bufs=2) as ppsb_pool,
            tc.tile_pool(name="r4", bufs=1, space="PSUM") as r4_pool,
            tc.tile_pool(name="r4sb", bufs=2) as r4sb_pool,
            tc.tile_pool(name="scr", bufs=2, space="DRAM") as scr_pool,
            tc.tile_pool(name="sk4", bufs=2) as sk4_pool,
            tc.tile_pool(name="out4", bufs=1, space="PSUM") as out4_pool,
            tc.tile_pool(name="outsb", bufs=2) as outsb_pool,
        ):
            c32 = consts.tile([128, 32], BF16, tag="c32")
            nc.sync.dma_start(c32[:], c32_dram.ap())
            bd4 = consts.tile([128, 4], BF16, tag="bd4")
            nc.sync.dma_start(bd4[:], bd4_dram.ap())

            for grp in range(GROUPS):
                r4 = r4_pool.tile([128, 1024], F32, tag="r4")
                sk4 = sk4_pool.tile([128, 516], BF16, tag="sk4")
                scrs = []
                for s in range(4):
                    pair = grp * 4 + s
                    # ---- stage 0: loads ----
                    x2c = x2p.tile([64, 128], BF16, tag="x2c")
                    nc.sync.dma_start(
                        x2c[:], d2.ap()[pair].rearrange("(q p) -> q p", p=128)
                    )
                    a5 = apool.tile([64, A5LEN], BF16, tag="a5")
                    nc.sync.dma_start(
                        a5[:],
                        bass.AP(
                            d1p,
                            pair * X1LEN + XPAD - A5OFF,
                            [[128, 64], [1, A5LEN]],
                        ),
                    )

                    # ---- stage 1: 22+ matmuls -> PP ----
                    pp = pp_pool.tile([128, 1024], F32, tag="pp")
                    for (b, g, clo, chi, wlo) in MM_BLOCKS:
                        j0 = 128 * g + clo + A5OFF
                        nc.tensor.matmul(
                            pp[32 * b:32 * b + 32, wlo:wlo + (chi - clo)],
                            x2c[:, 32 * b:32 * b + 32],
                            a5[:, j0:j0 + (chi - clo)],
                            start=True,
                            stop=True,
                            tile_position=(0, 32 * b),
                        )

                    # ---- stage 2: drain PP -> SBUF (DVE + ACT split) ----
                    ppsb = ppsb_pool.tile([128, W], BF16, tag="ppsb")
                    nc.vector.tensor_copy(ppsb[:, 0:273], pp[:, 0:273])
                    nc.scalar.copy(ppsb[:, 273:W], pp[:, 273:W])

                    # ---- stage 3: class reduce (4 beta -> 1) ----
                    for n0, n1 in ((0, 512), (512, W)):
                        nc.tensor.matmul(
                            r4[32 * s:32 * s + 32, n0:n1],
                            c32[:],
                            ppsb[:, n0:n1],
                            start=True,
                            stop=True,
                            tile_position=(0, 32 * s),
                        )

                # ---- stage 4: drain R4 ----
                r4sb = r4sb_pool.tile([128, W], BF16, tag="r4sb")
                nc.vector.tensor_copy(r4sb[:, 0:273], r4[:, 0:273])
                nc.scalar.copy(r4sb[:, 273:W], r4[:, 273:W])

                # ---- stage 5+6: DRAM round trip with skewed re-read ----
                for s in range(4):
                    scr = scr_pool.tile([32, SCR_STRIDE], BF16, tag="scr")
                    scrs.append(scr)
                    nc.sync.dma_start(scr[:, 0:W], r4sb[32 * s:32 * s + 32, 0:W])
                for s in range(4):
                    scr = scrs[s]
                    base = scr[:]
                    skew_src = bass.AP(
                        base.tensor,
                        base.offset,
                        [[SCR_STRIDE + 1, 32], [1, 516]],
                    )
                    nc.sync.dma_start(sk4[32 * s:32 * s + 32, :], skew_src)

                # ---- stage 7: sum 32 phases per pair (4 pairs at once) ----
                out4 = out4_pool.tile([4, 1024], F32, tag="out4")
                for n0, n1 in ((0, 512), (512, 516)):
                    nc.tensor.matmul(
                        out4[:, n0:n1],
                        bd4[:],
                        sk4[:, n0:n1],
                        start=True,
                        stop=True,
                    )

                # ---- stage 8+9: drain + store ----
                outsb = outsb_pool.tile([4, OUT_LEN], F32, tag="outsb")
                nc.vector.tensor_copy(outsb[:], out4[:, 0:OUT_LEN])
                nc.sync.dma_start(out.ap()[grp * 4:grp * 4 + 4, :], outsb[:])

    return nc


_NC_CACHE = {}


def _get_nc():
    if "nc" not in _NC_CACHE:
        nc = bacc.Bacc("TRN2", target_bir_lowering=False, debug=False)
        _build(nc)
        nc.compile()
        _NC_CACHE["nc"] = nc
    return _NC_CACHE["nc"]


def _make_in_maps(data1: np.ndarray, data2: np.ndarray):
    data1 = np.asarray(data1, dtype=np.float32).astype(ml_dtypes.bfloat16)
    data2 = np.asarray(data2, dtype=np.float32).astype(ml_dtypes.bfloat16)
    in_maps = []
    for k in range(N_CORES):
        d1 = data1[k * NB_PER_CORE:(k + 1) * NB_PER_CORE].reshape(PAIRS, NT)
        d2 = data2[k * NB_PER_CORE:(k + 1) * NB_PER_CORE].reshape(PAIRS, NT)
        d1p = np.zeros((PAIRS, X1LEN), ml_dtypes.bfloat16)
        d1p[:, XPAD:XPAD + NT] = d1
        in_maps.append({"d1p": d1p, "d2": np.ascontiguousarray(d2)})
    return in_maps


def run(data1: np.ndarray, data2: np.ndarray, trace: bool = False):
    nc = _get_nc()
    in_maps = _make_in_maps(data1, data2)
    res = run_bass_kernel_spmd(
        nc, in_maps, core_ids=list(range(N_CORES)), trace=trace
    )
    outs = [res.results[k]["out"].reshape(NB_PER_CORE, NCH, OUT_LEN)
            for k in range(N_CORES)]
    full = np.concatenate(outs, axis=0).astype(np.float32)
    return full, res


def kernel(data1: np.ndarray, data2: np.ndarray) -> np.ndarray:
    full, _ = run(data1, data2, trace=False)
    return full



# revision 2
# speedup vs baseline: 1.7126x; 1.7126x over previous
"""Trainium2 Bass kernel: grouped (depthwise) time-domain cross-correlation.

Problem: data1, data2 [nb=32, nc=64, nt=8192] f32.
out[b,c,l] = sum_t data2[b,c,t] * data1[b,c, t + l - 257]  (data1 zero-padded),
l in [0, 515).   (== F.conv1d groups=nb*nc, padding=257)

Sharding: data-parallel over nb across 8 NeuronCores (4 nb-rows = 256 pairs
each).

Per-core algorithm (per pair), v2:
  Stage 1 (PE):  stationary B[q,p] = x2[128q+p] (K=64 chunks, col-sliced per
    32-phase-class beta b); moving = a5[q, j] = x1[128q + j - 257] (fat
    window, j in [0, 644)).  PP[32b+m, w] = sum_q x2[128q+32b+m] *
    x1[128q + w + 32b - 257], i.e. moving slice j = w + 32b.  Per beta just
    2 matmuls tiling w in [0,512) (psum bank0) and [512, 548) (bank1):
    8 matmuls/pair, each PSUM cell written exactly once.  PP[32b+m, w]
    holds the lag-partial l = w - m for chunk-position 32b+m.
  Stage 2 (DVE+ACT): drain PP -> SBUF bf16.
  Stage 3 (PE): constant class matrix C32[p,m] = (p%32==m) reduces the 4
    beta-classes:  R[m,w] = sum_b PP[32b+m, w]  (col-tiled across the 4
    pairs of a group).
  Stage 4-6: R -> DRAM scratch (row stride 576, one batched DMA per group)
    -> skewed re-read with per-row offset m (stride 577) gives
    SK[m,l] = R[m, l+m]  (DRAM-side access patterns may step 1 element per
    partition, on-chip ones cannot).
  Stage 7 (PE): constant block-ones BD4[p,j] = (p//32==j) sums the 32 phases
    for 4 pairs at once: out4[j,l] = sum_m SK_j[m,l].
"""

import ml_dtypes
import numpy as np

import concourse.bacc as bacc
import concourse.bass as bass
import concourse.mybir as mybir
import concourse.tile as tile
from concourse.bass_utils import run_bass_kernel_spmd

# ---------------- problem constants (hardcoded per spec) ----------------
NB, NCH, NT = 32, 64, 8192
N_CORES = 8
NB_PER_CORE = NB // N_CORES          # 4
PAIRS = NB_PER_CORE * NCH            # 256 pairs per core
OUT_LEN = 515                        # 2*256 + 3
W = 548                              # w = l + m grid
XPAD = 384                           # x1 index range [-384, 8575]
X1LEN = NT + 2 * XPAD                # 8960
SCR_STRIDE = 576                     # scratch row stride (elements)
GROUPS = PAIRS // 4                  # 64 groups of 4 pairs

F32 = mybir.dt.float32
BF16 = mybir.dt.bfloat16

# fat x1 moving window: a5[q, j] = x1[128q + j - A5OFF], j in [0, A5LEN)
A5OFF = 257
A5LEN = 644

# stage-1 w-blocks (per beta): [0, 512) -> psum bank0, [512, 548) -> bank1
S1_BLOCKS = ((0, 512), (512, W))


def _consts():
    c32 = np.zeros((128, 32), np.float32)
    for p in range(128):
        c32[p, p % 32] = 1.0
    bd4 = np.zeros((128, 4), np.float32)
    for p in range(128):
        bd4[p, p // 32] = 1.0
    return c32, bd4


def _build(nc: bass.Bass):
    d1p = nc.dram_tensor("d1p", [PAIRS, X1LEN], BF16, kind="ExternalInput")
    d2 = nc.dram_tensor("d2", [PAIRS, NT], BF16, kind="ExternalInput")
    out = nc.dram_tensor("out", [PAIRS, OUT_LEN], F32, kind="ExternalOutput")

    c32_np, bd4_np = _consts()
    c32_dram = nc.inline_tensor(c32_np.astype(ml_dtypes.bfloat16), name="c32")
    bd4_dram = nc.inline_tensor(bd4_np.astype(ml_dtypes.bfloat16), name="bd4")

    with tile.TileContext(nc) as tc:
        with (
            tc.tile_pool(name="consts", bufs=1) as consts,
            tc.tile_pool(name="x2p", bufs=2) as x2p,
            tc.tile_pool(name="apool", bufs=2) as apool,
            tc.tile_pool(name="pp", bufs=2, space="PSUM") as pp_pool,
            tc.tile_pool(name="ppsb", bufs=2) as ppsb_pool,
            tc.tile_pool(name="r4", bufs=1, space="PSUM") as r4_pool,
            tc.tile_pool(name="r4sb", bufs=2) as r4sb_pool,
            tc.tile_pool(name="scr", bufs=2, space="DRAM") as scr_pool,
            tc.tile_pool(name="sk4", bufs=2) as sk4_pool,
            tc.tile_pool(name="out4", bufs=1, space="PSUM") as out4_pool,
            tc.tile_pool(name="outsb", bufs=2) as outsb_pool,
        ):
            c32 = consts.tile([128, 32], BF16, tag="c32")
            nc.sync.dma_start(c32[:], c32_dram.ap())
            bd4 = consts.tile([128, 4], BF16, tag="bd4")
            nc.sync.dma_start(bd4[:], bd4_dram.ap())

            for grp in range(GROUPS):
                p0 = grp * 4

                # ---- batched loads for the 4-pair group ----
                # x2c4[q, 128s + p] = x2_pair(p0+s)[128q + p]
                x2c4 = x2p.tile([64, 512], BF16, tag="x2c4")
                nc.sync.dma_start(
                    x2c4[:],
                    bass.AP(d2, p0 * NT, [[128, 64], [NT, 4], [1, 128]]),
                )
                # a54[q, 644s + j] = x1_pair(p0+s)[128q + j - A5OFF]
                a54 = apool.tile([64, 4 * A5LEN], BF16, tag="a54")
                nc.sync.dma_start(
                    a54[:],
                    bass.AP(
                        d1p,
                        p0 * X1LEN + XPAD - A5OFF,
                        [[128, 64], [X1LEN, 4], [1, A5LEN]],
                    ),
                )

                r4 = r4_pool.tile([128, 1024], F32, tag="r4")
                for s in range(4):
                    # ---- stage 1: 8 matmuls -> PP ----
                    pp = pp_pool.tile([128, 1024], F32, tag="pp")
                    for b in range(4):
                        for (wa, wb) in S1_BLOCKS:
                            ja = s * A5LEN + wa + 32 * b
                            nc.tensor.matmul(
                                pp[32 * b:32 * b + 32, wa:wb],
                                x2c4[:, 128 * s + 32 * b:128 * s + 32 * b + 32],
                                a54[:, ja:ja + (wb - wa)],
                                start=True,
                                stop=True,
                                tile_position=(0, 32 * b),
                            )

                    # ---- stage 2: drain PP -> SBUF (DVE + ACT split) ----
                    ppsb = ppsb_pool.tile([128, W], BF16, tag="ppsb")
                    nc.vector.tensor_copy(ppsb[:, 0:274], pp[:, 0:274])
                    nc.scalar.copy(ppsb[:, 274:W], pp[:, 274:W])

                    # ---- stage 3: class reduce (4 beta -> 1) ----
                    for n0, n1 in ((0, 512), (512, W)):
                        nc.tensor.matmul(
                            r4[32 * s:32 * s + 32, n0:n1],
                            c32[:],
                            ppsb[:, n0:n1],
                            start=True,
                            stop=True,
                            tile_position=(0, 32 * s),
                        )

                # ---- stage 4: drain R4 ----
                r4sb = r4sb_pool.tile([128, W], BF16, tag="r4sb")
                nc.vector.tensor_copy(r4sb[:, 0:274], r4[:, 0:274])
                nc.scalar.copy(r4sb[:, 274:W], r4[:, 274:W])

                # ---- stage 5+6: DRAM round trip with skewed re-read ----
                scr = scr_pool.tile([128, SCR_STRIDE], BF16, tag="scr")
                nc.sync.dma_start(scr[:, 0:W], r4sb[:, 0:W])
                sk4 = sk4_pool.tile([128, 516], BF16, tag="sk4")
                base = scr[:]
                nc.sync.dma_start(
                    sk4[:],
                    bass.AP(
                        base.tensor,
                        base.offset,
                        [[32 * SCR_STRIDE, 4], [SCR_STRIDE + 1, 32], [1, 516]],
                    ),
                )

                # ---- stage 7: sum 32 phases per pair (4 pairs at once) ----
                out4 = out4_pool.tile([4, 1024], F32, tag="out4")
                for n0, n1 in ((0, 512), (512, 516)):
                    nc.tensor.matmul(
                        out4[:, n0:n1],
                        bd4[:],
                        sk4[:, n0:n1],
                        start=True,
                        stop=True,
                    )

                # ---- stage 8+9: drain + store ----
                outsb = outsb_pool.tile([4, OUT_LEN], F32, tag="outsb")
                nc.vector.tensor_copy(outsb[:], out4[:, 0:OUT_LEN])
                nc.sync.dma_start(out.ap()[p0:p0 + 4, :], outsb[:])

    return nc


_NC_CACHE = {}


def _get_nc():
    if "nc" not in _NC_CACHE:
        nc = bacc.Bacc("TRN2", target_bir_lowering=False, debug=False)
        _build(nc)
        nc.compile()
        _NC_CACHE["nc"] = nc
    return _NC_CACHE["nc"]


def _make_in_maps(data1: np.ndarray, data2: np.ndarray):
    data1 = np.asarray(data1, dtype=np.float32).astype(ml_dtypes.bfloat16)
    data2 = np.asarray(data2, dtype=np.float32).astype(ml_dtypes.bfloat16)
    in_maps = []
    for k in range(N_CORES):
        d1 = data1[k * NB_PER_CORE:(k + 1) * NB_PER_CORE].reshape(PAIRS, NT)
        d2 = data2[k * NB_PER_CORE:(k + 1) * NB_PER_CORE].reshape(PAIRS, NT)
        d1p = np.zeros((PAIRS, X1LEN), ml_dtypes.bfloat16)
        d1p[:, XPAD:XPAD + NT] = d1
        in_maps.append({"d1p": d1p, "d2": np.ascontiguousarray(d2)})
    return in_maps


def run(data1: np.ndarray, data2: np.ndarray, trace: bool = False):
    nc = _get_nc()
    in_maps = _make_in_maps(data1, data2)
    res = run_bass_kernel_spmd(
        nc, in_maps, core_ids=list(range(N_CORES)), trace=trace
    )
    outs = [res.results[k]["out"].reshape(NB_PER_CORE, NCH, OUT_LEN)
            for k in range(N_CORES)]
    full = np.concatenate(outs, axis=0).astype(np.float32)
    return full, res


def kernel(data1: np.ndarray, data2: np.ndarray) -> np.ndarray:
    full, _ = run(data1, data2, trace=False)
    return full
